# revision 12
# baseline (speedup 1.0000x reference)
"""Trainium2 Bass kernel for nn_DecoderSeq: LSTM decoder, B=256, E=512,
HID=1024, 256 steps, projection to 2 outputs per step.

Strategy: pure data-parallel over batch (8 cores x 32 rows, no cross-core
communication - the recurrence is per-row). Per core, everything is kept
"gate/hid-major": the LSTM state is stored transposed as [hid, batch] so
the recurrent matmul gates_T = W_hh @ h_T runs with W_hh tiles as the
stationary operand and h_T as the moving operand, activations get full
128-partition utilization, biases are per-partition scalars, and no
transposes are ever needed (h_T slices feed the next step's matmuls
directly).

W_hh and h are bf16 (fp32 matmuls run at 1/4 rate on the PE and fp32
weight loads at 1/4 FWL rate; bf16 keeps the recurrence within ~3e-3 of
the fp32 reference). Accumulation stays fp32 in PSUM, x_pre/c/gates stay
fp32.

Layouts (free index always (block, batch)):
  w_hh packed  [128, 8*4096]: col k*4096+j  = W_hh.T[128k+p, j]
  w_ih packed  [128, 4*4096]: col e*4096+j  = W_ih.T[128e+p, j]
  z packed     [128, 4*32]:   col 32e+b     = z[b, 128e+p]
  bias packed  [128, 32]:     col jj        = (b_ih+b_hh)[128jj+p]
  w_proj packed[128, 16]:     col 2k+d      = W_proj.T[128k+p, d]
  gates/x_pre  [128, 32*32]:  col 32jj+b    = gate row 128jj+p, batch b
  h,c,i,f,g,o  [128, 8*32]:   col 32blk+b   = hid 128blk+p, batch b
  ys           [2, 256*32]:   col 32t+b     = output[b, t, d=partition]
"""

import sys

sys.path.insert(0, "/opt/trn_rl_repo")

import numpy as np

B, E, HID, STEPS = 256, 512, 1024, 256
N_CORES = 8
BL = B // N_CORES  # 32 batch rows per core
G4 = 4 * HID  # 4096 gate rows
KC = HID // 128  # 8 k-chunks of the hidden contraction
EC = E // 128  # 4 e-chunks of the input contraction
JB = G4 // 128  # 32 gate-row blocks
UNROLL = 4  # steps emitted per For_i iteration
WDTYPE = "fp8"  # "bf16" | "fp8": dtype of W_hh + the gates-matmul h operand

_CACHE = {}


def _split_multi_waits(nc, mybir):
    # This walrus build rejects >1 sync wait per instruction; hoist extra
    # waits onto single-wait NoOps just before, on the same engine.
    n = 0
    for fn in nc.m.functions:
        for bb in fn.blocks:
            new_insts = []
            for inst in bb.instructions:
                si = inst.sync_info
                if si is not None and si.on_wait is not None and len(si.on_wait) > 1:
                    waits = list(si.on_wait)
                    for w in waits[:-1]:
                        new_insts.append(
                            mybir.InstNoOp(
                                name=f"{inst.name}_ws{n}",
                                engine=inst.engine,
                                sync_info=mybir.SyncInfo(on_wait=[w], on_update=[]),
                            )
                        )
                        n += 1
                    si.on_wait = [waits[-1]]
                new_insts.append(inst)
            bb.instructions[:] = new_insts
    return n


def _build(loop_mult=1, skip_prologue=False, static_ys=False, wdtype=None):
    import concourse.bass as bass
    import concourse.mybir as mybir
    import concourse.tile as tile
    from concourse.bass import ds, ts

    F32 = mybir.dt.float32
    AF = mybir.ActivationFunctionType
    ALU = mybir.AluOpType

    nc = bass.Bass("TRN2", target_bir_lowering=False, debug=False)

    BF16 = mybir.dt.bfloat16
    if wdtype is None:
        wdtype = WDTYPE
    WDT = mybir.dt.float8e4 if wdtype == "fp8" else BF16
    whh_ap = nc.dram_tensor("whh", [128, KC * G4], WDT, kind="ExternalInput").ap()
    wih_ap = nc.dram_tensor("wih", [128, EC * G4], F32, kind="ExternalInput").ap()
    z_ap = nc.dram_tensor("z", [128, EC * BL], F32, kind="ExternalInput").ap()
    bias_ap = nc.dram_tensor("bias", [128, JB], F32, kind="ExternalInput").ap()
    wproj_ap = nc.dram_tensor("wproj", [128, 2 * KC], BF16, kind="ExternalInput").ap()
    bproj_ap = nc.dram_tensor("bproj", [2, 1], F32, kind="ExternalInput").ap()
    total_steps = STEPS * loop_mult
    ys_steps = STEPS if static_ys else total_steps
    ys_ap = nc.dram_tensor("ys", [2, ys_steps * BL], F32, kind="ExternalOutput").ap()

    with tile.TileContext(nc) as tc:
        with (
            tc.tile_pool(name="consts", bufs=1) as consts,
            tc.tile_pool(name="state", bufs=1) as state,
            tc.tile_pool(name="work", bufs=2) as work,
            tc.tile_pool(name="psum", bufs=2, space="PSUM") as psum_pool,
            tc.tile_pool(name="psum_p", bufs=2, space="PSUM") as psum_proj,
        ):
            z_sb = consts.tile([128, EC * BL], F32)
            nc.gpsimd.dma_start(z_sb[:], z_ap[:])
            bias_sb = consts.tile([128, JB], F32)
            nc.gpsimd.dma_start(bias_sb[:], bias_ap[:])
            wproj_sb = consts.tile([128, 2 * KC], BF16)
            nc.gpsimd.dma_start(wproj_sb[:], wproj_ap[:])
            bproj_sb = consts.tile([2, 1], F32)
            nc.gpsimd.dma_start(bproj_sb[:], bproj_ap[:])

            x_sb = state.tile([128, JB * BL], F32)  # x_pre, gate-major
            h_sb = state.tile([128, KC * BL], WDT)
            h16_sb = state.tile([128, KC * BL], BF16)  # proj-path copy
            c_sb = state.tile([128, KC * BL], F32)
            ys_sb = state.tile([2, ys_steps * BL], F32)
            nc.vector.memset(h_sb[:], 0.0)
            nc.vector.memset(h16_sb[:], 0.0)
            nc.vector.memset(c_sb[:], 0.0)

            # ---- prologue: x_pre = W_ih @ z_T + (b_ih + b_hh), gate-major.
            # Stream W_ih from DRAM in 4 chunks through a transient pool.
            # NOTE: PSUM accumulation groups (start..stop) must be emitted
            # contiguously per psum region - interleaving groups across jj
            # produced wrong results on HW. So: all e for one jj, then next.
            if skip_prologue:
                # Timing-only builds: the prologue cancels out of the
                # loop-mult differencing, and its W_ih staging doesn't fit
                # next to the 3x-sized ys buffer.
                nc.vector.memset(x_sb[:], 0.0)
            else:
                px = [
                    psum_pool.tile([128, 16 * BL], F32, tag="gpsum0", name=f"px{i}")
                    for i in range(2)
                ]
                with tc.tile_pool(name="wih_stream", bufs=1) as wih_pool:
                    wih_sbs = []
                    for e in range(EC):
                        wih_sb = wih_pool.tile(
                            [128, G4], F32, tag=f"wih{e}", name=f"wih{e}"
                        )
                        nc.gpsimd.dma_start(wih_sb[:], wih_ap[:, ts(e, G4)])
                        wih_sbs.append(wih_sb)
                    for jj in range(JB):
                        for e in range(EC):
                            nc.tensor.matmul(
                                px[jj // 16][:, ts(jj % 16, BL)],
                                wih_sbs[e][:, ts(jj, 128)],
                                z_sb[:, ts(e, BL)],
                                start=(e == 0),
                                stop=(e == EC - 1),
                            )
                    for jj in range(JB):
                        nc.scalar.activation(
                            x_sb[:, ts(jj, BL)],
                            px[jj // 16][:, ts(jj % 16, BL)],
                            AF.Identity,
                            bias=bias_sb[:, jj : jj + 1],
                        )

            # ---- resident recurrent weights (64 KB/partition, bf16)
            whh_sb = consts.tile([128, KC * G4], WDT)
            nc.gpsimd.dma_start(whh_sb[:], whh_ap[:])

            # ---- the 256-step recurrence.
            # Gate blocks are host-permuted to (i, f, o, g): blocks 0-23 are
            # the three sigmoid gates (one big ACT op, overlaps the g-block
            # matmuls), blocks 24-31 are g (tanh). f*c also fires during the
            # g matmuls, shortening the serial tail to add+tanh+2 muls.
            def step(iv):
                # iv: scalar step index (ScalarValue or int)
                ps0 = psum_pool.tile([128, 24 * BL], F32, tag="gpsum0")
                for jj in range(24):
                    for k in range(KC):
                        off = k * G4 + jj * 128
                        nc.tensor.matmul(
                            ps0[:, ts(jj, BL)],
                            whh_sb[:, off : off + 128],
                            h_sb[:, ts(k, BL)],
                            start=(k == 0),
                            stop=(k == KC - 1),
                        )
                ps1 = psum_pool.tile([128, 8 * BL], F32, tag="gpsum1")
                for jj8 in range(8):
                    jj = 24 + jj8
                    for k in range(KC):
                        off = k * G4 + jj * 128
                        nc.tensor.matmul(
                            ps1[:, ts(jj8, BL)],
                            whh_sb[:, off : off + 128],
                            h_sb[:, ts(k, BL)],
                            start=(k == 0),
                            stop=(k == KC - 1),
                        )

                ifo_pre = work.tile([128, 24 * BL], F32, tag="ifopre")
                nc.vector.scalar_tensor_tensor(
                    ifo_pre[:], ps0[:], 1.0, x_sb[:, : 24 * BL], ALU.mult, ALU.add
                )
                ifo = work.tile([128, 24 * BL], F32, tag="ifo")
                nc.scalar.activation(ifo[:], ifo_pre[:], AF.Sigmoid)
                fc = work.tile([128, 8 * BL], F32, tag="fc")
                nc.vector.tensor_mul(fc[:], ifo[:, 8 * BL : 16 * BL], c_sb[:])

                g_pre = work.tile([128, 8 * BL], F32, tag="gpre")
                nc.vector.scalar_tensor_tensor(
                    g_pre[:], ps1[:], 1.0, x_sb[:, 24 * BL :], ALU.mult, ALU.add
                )
                g_act = work.tile([128, 8 * BL], F32, tag="gact")
                nc.scalar.activation(g_act[:], g_pre[:], AF.Tanh)

                ig = work.tile([128, 8 * BL], F32, tag="ig")
                nc.vector.tensor_mul(ig[:], ifo[:, : 8 * BL], g_act[:])
                nc.vector.tensor_add(c_sb[:], ig[:], fc[:])
                tc_t = work.tile([128, 8 * BL], F32, tag="tct")
                nc.scalar.activation(tc_t[:], c_sb[:], AF.Tanh)
                nc.vector.tensor_mul(h_sb[:], ifo[:, 16 * BL :], tc_t[:])
                nc.vector.tensor_mul(h16_sb[:], ifo[:, 16 * BL :], tc_t[:])

                # projection: ys[:, t] = tanh(W_proj @ h + b_proj)
                pp = psum_proj.tile([2, BL], F32, tag="ppsum")
                for k in range(KC):
                    nc.tensor.matmul(
                        pp[:],
                        wproj_sb[:, ts(k, 2)],
                        h16_sb[:, ts(k, BL)],
                        start=(k == 0),
                        stop=(k == KC - 1),
                    )
                ys_slice = ys_sb[:, ts(0, BL)] if static_ys else ys_sb[:, ds(iv * BL, BL)]
                nc.scalar.activation(
                    ys_slice,
                    pp[:],
                    AF.Tanh,
                    bias=bproj_sb[:, 0:1],
                )

            with tc.For_i(
                0, total_steps // UNROLL, hint_engines=(mybir.EngineType.PE,)
            ) as it:
                for u in range(UNROLL):
                    step(it * UNROLL + u)

            nc.gpsimd.dma_start(ys_ap[:], ys_sb[:])

    _split_multi_waits(nc, mybir)
    return nc


# Host-side gate-row permutation: reference order (i, f, g, o) -> kernel
# order (i, f, o, g) so the three sigmoid gates are contiguous.
_PERM = np.concatenate(
    [np.arange(0, 2 * HID), np.arange(3 * HID, 4 * HID), np.arange(2 * HID, 3 * HID)]
)


def _pack_inputs(z, W_ih, W_hh, b_ih, b_hh, W_proj, b_proj):
    import ml_dtypes

    W_hh = W_hh[_PERM]
    W_ih = W_ih[_PERM]
    b_sum = (b_ih + b_hh)[_PERM]

    wnp = ml_dtypes.float8_e4m3 if WDTYPE == "fp8" else ml_dtypes.bfloat16
    whh_pk = np.ascontiguousarray(
        W_hh.T.reshape(KC, 128, G4).transpose(1, 0, 2).reshape(128, KC * G4)
    ).astype(wnp)
    wih_pk = np.ascontiguousarray(
        W_ih.T.reshape(EC, 128, G4).transpose(1, 0, 2).reshape(128, EC * G4)
    ).astype(np.float32)
    bias_pk = np.ascontiguousarray(b_sum.reshape(JB, 128).T).astype(np.float32)
    wproj_pk = np.ascontiguousarray(
        W_proj.T.reshape(KC, 128, 2).transpose(1, 0, 2).reshape(128, 2 * KC)
    ).astype(ml_dtypes.bfloat16)
    bproj_pk = np.ascontiguousarray(b_proj.reshape(2, 1)).astype(np.float32)

    in_maps = []
    for c in range(N_CORES):
        z_loc = z[c * BL : (c + 1) * BL]  # [32, 512]
        z_pk = np.ascontiguousarray(
            z_loc.T.reshape(EC, 128, BL).transpose(1, 0, 2).reshape(128, EC * BL)
        ).astype(np.float32)
        in_maps.append(
            {
                "whh": whh_pk,
                "wih": wih_pk,
                "z": z_pk,
                "bias": bias_pk,
                "wproj": wproj_pk,
                "bproj": bproj_pk,
            }
        )
    return in_maps


def _get_nc():
    if "nc" not in _CACHE:
        _CACHE["nc"] = _build()
    return _CACHE["nc"]


def kernel(z, W_ih, W_hh, b_ih, b_hh, W_proj, b_proj, H):
    from concourse.bass_utils import run_bass_kernel_spmd

    z = np.asarray(z, dtype=np.float32)
    W_ih = np.asarray(W_ih, dtype=np.float32)
    W_hh = np.asarray(W_hh, dtype=np.float32)
    b_ih = np.asarray(b_ih, dtype=np.float32)
    b_hh = np.asarray(b_hh, dtype=np.float32)
    W_proj = np.asarray(W_proj, dtype=np.float32)
    b_proj = np.asarray(b_proj, dtype=np.float32)
    assert int(H) == STEPS and z.shape == (B, E)

    nc = _get_nc()

    in_maps = _pack_inputs(z, W_ih, W_hh, b_ih, b_hh, W_proj, b_proj)
    res = run_bass_kernel_spmd(nc, in_maps, list(range(N_CORES)))

    out = np.empty((B, STEPS, 2), dtype=np.float32)
    for c in range(N_CORES):
        ys = res.results[c]["ys"]  # [2, STEPS*BL]
        out[c * BL : (c + 1) * BL] = ys.reshape(2, STEPS, BL).transpose(2, 1, 0)
    return out
